# revision 2
# baseline (speedup 1.0000x reference)
"""Chamfer loss kernel for 8x TRN2 NeuronCores (Bass/Tile).

Strategy (data-parallel over batch, one batch per core):
  Single matmul pass producing the FULL distance matrix in PSUM fp32:
    dist[m,n] = t2[m] + p2[n] - 2 t.p  via K=15 bf16 hi/lo matmul
    (aug moving b = [p,p2,1] hi|lo|hi, aug weights a = [-2t,1,t2] hi|hi|lo).
  ACT engine converts each PSUM tile to SBUF bf16 (only engine with spare
  throughput). DVE consumes bf16 at 2x rate (2x_1p) via tensor_tensor min:
    pass A (min over n per m): per-m-tile binary fold chain -> [128,128]
      remnant per m-tile, stacked, folded at the end.
    pass B (min over m per n): running elementwise min into accB.
  Pass-B cross-partition finish: PE transposes of accB + TT folds.
  m-tiles processed in PAIRS via 3D access patterns to halve DVE
  instruction count.
"""
import numpy as np
import ml_dtypes
from contextlib import ExitStack

import jax
from jax.sharding import Mesh, PartitionSpec
from jax.experimental.shard_map import shard_map

import concourse.bacc as bacc
import concourse.tile as tile
import concourse.mybir as mybir
import concourse.bass as bass
from concourse.bass2jax import (
    _bass_exec_p,
    install_neuronx_cc_hook,
    partition_id_tensor,
)

N_CORES = 8
F32 = mybir.dt.float32
BF16 = mybir.dt.bfloat16
MIN = mybir.AluOpType.min
ADD = mybir.AluOpType.add
X = mybir.AxisListType.X


def build_nc(npts=8192, reps=1):
    """One core's kernel: pred/target [npts,3] f32 + ident [128,128] bf16;
    output res [128, 2] f32 = [colsum minA, colsum minB]."""
    P = 128
    Q = npts // P            # 64 points per partition (natural layout)
    NT = npts // 128         # 64 m-tiles
    K = 15

    nc = bacc.Bacc("TRN2", target_bir_lowering=False, debug=False)
    pred = nc.dram_tensor("pred", [npts, 3], F32, kind="ExternalInput")
    targ = nc.dram_tensor("target", [npts, 3], F32, kind="ExternalInput")
    ident = nc.dram_tensor("ident", [128, 128], BF16, kind="ExternalInput")
    out = nc.dram_tensor("res", [128, 2], F32, kind="ExternalOutput")

    with tile.TileContext(nc) as tc, ExitStack() as ctx:
        sb = ctx.enter_context(tc.tile_pool(name="sb", bufs=1))

        # ---- load natural layouts -------------------------------------
        pnat = sb.tile([P, Q * 3], F32)
        nc.sync.dma_start(pnat[:], pred.ap().rearrange("(p q) d -> p (q d)", p=P))
        tnat = sb.tile([P, Q * 3], F32)
        nc.sync.dma_start(tnat[:], targ.ap().rearrange("(p q) d -> p (q d)", p=P))
        idt = sb.tile([128, 128], BF16)
        nc.sync.dma_start(idt[:], ident.ap())

        pv = pnat[:].rearrange("p (q d) -> p q d", d=3)
        tv = tnat[:].rearrange("p (q d) -> p q d", d=3)

        # ---- squared norms (fp32, natural layout) ---------------------
        sq = sb.tile([P, Q * 3], F32)
        sqv = sq[:].rearrange("p (q d) -> p q d", d=3)
        p2 = sb.tile([P, Q], F32)
        p2v = p2[:].rearrange("p (q d) -> p q d", d=1)
        t2 = sb.tile([P, Q], F32)
        t2v = t2[:].rearrange("p (q d) -> p q d", d=1)

        nc.vector.tensor_mul(sq[:], pnat[:], pnat[:])
        nc.vector.tensor_add(p2v[:, :, 0:1], sqv[:, :, 0:1], sqv[:, :, 1:2])
        nc.vector.tensor_add(p2v[:, :, 0:1], p2v[:, :, 0:1], sqv[:, :, 2:3])
        nc.vector.tensor_mul(sq[:], tnat[:], tnat[:])
        nc.vector.tensor_add(t2v[:, :, 0:1], sqv[:, :, 0:1], sqv[:, :, 1:2])
        nc.vector.tensor_add(t2v[:, :, 0:1], t2v[:, :, 0:1], sqv[:, :, 2:3])

        # ---- natural-layout K=15 assemblies (bf16 hi/lo) ---------------
        # moving b = [p(3), p2, 1]:   cols [b_hi(5) | b_lo(5) | b_hi(5)]
        # weights a = [-2t(3), 1, t2]: cols [a_hi(5) | a_hi(5) | a_lo(5)]
        SM = sb.tile([P, Q * K], BF16)
        SW = sb.tile([P, Q * K], BF16)
        sm = SM[:].rearrange("p (q c) -> p q c", c=K)
        sw = SW[:].rearrange("p (q c) -> p q c", c=K)

        # moving: b_hi at 0:5
        nc.vector.tensor_copy(sm[:, :, 0:3], pv[:])            # p_hi
        nc.vector.tensor_copy(sm[:, :, 3:4], p2v[:])           # p2_hi
        nc.vector.memset(sm[:, :, 4:5], 1.0)
        # b_lo at 5:10
        nc.vector.tensor_sub(sm[:, :, 5:8], pv[:], sm[:, :, 0:3])
        nc.vector.tensor_sub(sm[:, :, 8:9], p2v[:], sm[:, :, 3:4])
        nc.vector.memset(sm[:, :, 9:10], 0.0)
        # b_hi again at 10:15
        nc.vector.tensor_copy(sm[:, :, 10:13], sm[:, :, 0:3])
        nc.vector.tensor_copy(sm[:, :, 13:14], sm[:, :, 3:4])
        nc.vector.memset(sm[:, :, 14:15], 1.0)

        # weights: t_hi/t_lo first (exact -2x scaling of each half)
        th = sb.tile([P, Q * 3], BF16)
        thv = th[:].rearrange("p (q d) -> p q d", d=3)
        tl = sb.tile([P, Q * 3], BF16)
        tlv = tl[:].rearrange("p (q d) -> p q d", d=3)
        nc.vector.tensor_copy(thv[:], tv[:])                   # t_hi
        nc.vector.tensor_sub(tlv[:], tv[:], thv[:])            # t_lo
        # a_hi at 0:5 = [-2 t_hi, 1, t2_hi]
        nc.vector.tensor_scalar_mul(sw[:, :, 0:3], thv[:], -2.0)
        nc.vector.memset(sw[:, :, 3:4], 1.0)
        nc.vector.tensor_copy(sw[:, :, 4:5], t2v[:])           # t2_hi
        # a_hi again at 5:10
        nc.vector.tensor_copy(sw[:, :, 5:8], sw[:, :, 0:3])
        nc.vector.memset(sw[:, :, 8:9], 1.0)
        nc.vector.tensor_copy(sw[:, :, 9:10], sw[:, :, 4:5])
        # a_lo at 10:15 = [-2 t_lo, 0, t2_lo]
        nc.vector.tensor_scalar_mul(sw[:, :, 10:13], tlv[:], -2.0)
        nc.vector.memset(sw[:, :, 13:14], 0.0)
        nc.vector.tensor_sub(sw[:, :, 14:15], t2v[:], sw[:, :, 4:5])

        # ---- transpose to K-major stacked operands ---------------------
        # [128, npts]: rows 0:15 hold K-vectors; replicated at partition
        # bases 32/64/96 for 4-way row-packed matmuls via tile_position.
        mstk = sb.tile([128, npts], BF16, name="mstk")
        wstk = sb.tile([128, npts], BF16, name="wstk")

        n_rounds = npts // 2048
        with tc.tile_pool(name="tpsum", bufs=2, space="PSUM") as tps:
            for rnd in range(n_rounds):
                for (srcS, stk) in ((sm, mstk), (sw, wstk)):
                    tp = tps.tile([K, 16 * 128], BF16)
                    for j in range(16):
                        blk = rnd * 16 + j
                        nc.tensor.transpose(
                            tp[:, j * 128:(j + 1) * 128],
                            srcS[:, blk:blk + 1, 0:K],
                            idt[:],
                        )
                    f0 = rnd * 2048
                    nc.scalar.copy(stk[0:K, f0:f0 + 2048], tp[:])
        for stk in (mstk, wstk):
            for i in range(1, 4):
                nc.sync.dma_start(stk[32 * i:32 * i + K, :], stk[0:K, :])

        # ---- persistent accumulators / scratch -------------------------
        # chunk-interleaved pair layout: column c = X*1024 + e*512 + u where
        # X = g*4+j is the 512-wide n-chunk (n = X*512+u) and e picks the
        # m-tile half of the pair.  All big TT-mins are then contiguous 2D.
        accB2 = sb.tile([128, 2 * npts], BF16, name="accB2")
        accB2v = accB2[:].rearrange("p (x e u) -> p x e u", e=2, u=512)
        stackA = sb.tile([128, NT * 128], BF16, name="stackA")
        stackAv = stackA[:].rearrange("p (t e w) -> p t e w", e=2, w=128)
        minAc = sb.tile([128, NT], BF16)
        minBc = sb.tile([128, NT], BF16)
        minAf = sb.tile([128, NT], F32)
        minBf = sb.tile([128, NT], F32)
        res = sb.tile([128, 2], F32)

        dpool = ctx.enter_context(tc.tile_pool(name="dp", bufs=2))
        fpool = ctx.enter_context(tc.tile_pool(name="fp", bufs=2))

        # ---- main loop --------------------------------------------------
        rep_ctx = ExitStack()
        with tc.tile_pool(name="pp", bufs=2, space="PSUM") as pp, rep_ctx:
            if reps > 1:
                rep_ctx.enter_context(tc.For_i(0, reps, 1))
            if True:
                for mtp in range(NT // 2):      # m-tile pairs
                    if mtp == 0:
                        dst = accB2          # ACT writes straight into accB
                    else:
                        dst = dpool.tile([128, 2 * npts], BF16, name="dbuf")
                    dstv = dst[:].rearrange("p (x e u) -> p x e u", e=2, u=512)
                    for half in range(2):
                        mt = 2 * mtp + half
                        for g in range(4):
                            pt = pp.tile([128, 2048], F32, name="ptile")
                            for j in range(4):
                                n0 = g * 2048 + j * 512
                                nc.tensor.matmul(
                                    pt[:, j * 512:(j + 1) * 512],
                                    wstk[32 * j:32 * j + K,
                                         mt * 128:(mt + 1) * 128],
                                    mstk[32 * j:32 * j + K, n0:n0 + 512],
                                    start=True, stop=True,
                                    tile_position=(32 * j, 0),
                                )
                            nc.scalar.copy(
                                dstv[:, 4 * g:4 * (g + 1), half:half + 1, :],
                                pt[:].rearrange("p (j o u) -> p j o u",
                                                o=1, u=512))
                    # pass B: running elementwise min (pair-wide, contiguous)
                    if mtp > 0:
                        nc.vector.tensor_tensor(accB2[:], accB2[:], dst[:],
                                                op=MIN)
                    # pass A: binary fold chain, contiguous halving down to
                    # [128, 1024], then two 3D levels within (e,u)
                    fs = fpool.tile([128, npts], BF16, name="fsbuf")
                    nc.vector.tensor_tensor(
                        fs[:, 0:8192], dst[:, 0:8192], dst[:, 8192:16384],
                        op=MIN)
                    w = 4096
                    while w >= 1024:
                        nc.vector.tensor_tensor(
                            fs[:, 0:w], fs[:, 0:w], fs[:, w:2 * w], op=MIN)
                        w //= 2
                    fsv = fs[:].rearrange("p (x e u) -> p x e u", e=2, u=512)
                    # here x==1 region: [128, 1, 2, 512] -> fold u
                    nc.vector.tensor_tensor(
                        fsv[:, 0:1, :, 0:256], fsv[:, 0:1, :, 0:256],
                        fsv[:, 0:1, :, 256:512], op=MIN)
                    nc.vector.tensor_tensor(
                        stackAv[:, mtp:mtp + 1, :, :], fsv[:, 0:1, :, 0:128],
                        fsv[:, 0:1, :, 128:256], op=MIN)

                # ---- minA: fold stackA [128, 32, 2, 128] -> [128, 64] ---
                fa = fpool.tile([128, npts], BF16, name="fsbuf")
                fav = fa[:].rearrange("p (t e w) -> p t e w", e=2, w=128)
                nc.vector.tensor_tensor(
                    fav[:, 0:32, :, 0:64], stackAv[:, :, :, 0:64],
                    stackAv[:, :, :, 64:128], op=MIN)
                w = 32
                while w >= 1:
                    nc.vector.tensor_tensor(
                        fav[:, 0:32, :, 0:w], fav[:, 0:32, :, 0:w],
                        fav[:, 0:32, :, w:2 * w], op=MIN)
                    w //= 2
                nc.vector.tensor_copy(
                    minAf[:].rearrange("p (t e w) -> p t e w", e=2, w=1),
                    fav[:, 0:32, :, 0:1])
                nc.vector.tensor_reduce(res[:, 0:1], minAf[:], axis=X, op=ADD)

                # ---- pass B finish: merge e-halves, transpose, fold -----
                fb = fpool.tile([128, npts], BF16, name="fsbuf")
                mrg = fb[:, 0:npts]
                nc.vector.tensor_tensor(
                    mrg.rearrange("p (x o u) -> p x o u", o=1, u=512),
                    accB2v[:, :, 0:1, :], accB2v[:, :, 1:2, :], op=MIN)
                fc = fpool.tile([128, npts], BF16, name="fsbuf")
                for r in range(4):
                    tpbF = pp.tile([128, 2048], F32, name="ptile")
                    tpb = tpbF[:].bitcast(BF16)  # [128, 4096] bf16 view
                    for b in range(16):
                        c0 = (r * 16 + b) * 128
                        nc.tensor.transpose(
                            tpb[:, b * 128:(b + 1) * 128],
                            mrg[:, c0:c0 + 128],
                            idt[:],
                        )
                    nc.scalar.copy(fc[:, r * 2048:(r + 1) * 2048],
                                   tpb[:, 0:2048])
                fcv = fc[:].rearrange("p (t w) -> p t w", t=NT)  # [128,64,128]
                nc.vector.tensor_tensor(
                    fcv[:, :, 0:64], fcv[:, :, 0:64], fcv[:, :, 64:128], op=MIN)
                w = 32
                while w >= 2:
                    nc.vector.tensor_tensor(
                        fcv[:, :, 0:w], fcv[:, :, 0:w], fcv[:, :, w:2 * w],
                        op=MIN)
                    w //= 2
                nc.vector.tensor_tensor(
                    minBc[:].rearrange("p (t w) -> p t w", w=1),
                    fcv[:, :, 0:1], fcv[:, :, 1:2], op=MIN)
                nc.vector.tensor_copy(minBf[:], minBc[:])
                nc.vector.tensor_reduce(res[:, 1:2], minBf[:], axis=X, op=ADD)

        nc.sync.dma_start(out.ap(), res[:])

    nc.compile()
    return nc


# ----------------------------------------------------------------------
# Host-side runner with jit cache
# ----------------------------------------------------------------------
_CACHE = {}


def _make_callable(nc, n_cores):
    install_neuronx_cc_hook()
    partition_name = nc.partition_id_tensor.name if nc.partition_id_tensor else None

    in_names, out_names, out_avals, zero_outs = [], [], [], []
    for alloc in nc.m.functions[0].allocations:
        if not isinstance(alloc, mybir.MemoryLocationSet):
            continue
        name = alloc.memorylocations[0].name
        if alloc.kind == "ExternalInput":
            if name != partition_name:
                in_names.append(name)
        elif alloc.kind == "ExternalOutput":
            out_names.append(name)
            shape = tuple(alloc.tensor_shape)
            dtype = mybir.dt.np(alloc.dtype)
            out_avals.append(jax.core.ShapedArray(shape, dtype))
            zero_outs.append(np.zeros(shape, dtype))
    n_params = len(in_names)
    n_outs = len(out_avals)
    all_in_names = list(in_names) + list(out_names)
    if partition_name is not None:
        all_in_names.append(partition_name)

    def _body(*args):
        operands = list(args)
        if partition_name is not None:
            operands.append(partition_id_tensor())
        outs = _bass_exec_p.bind(
            *operands,
            out_avals=tuple(out_avals),
            in_names=tuple(all_in_names),
            out_names=tuple(out_names),
            lowering_input_output_aliases=(),
            sim_require_finite=True,
            sim_require_nnan=True,
            nc=nc,
        )
        return tuple(outs)

    devices = jax.devices()[:n_cores]
    mesh = Mesh(np.asarray(devices), ("core",))
    in_specs = (PartitionSpec("core"),) * (n_params + n_outs)
    out_specs = (PartitionSpec("core"),) * n_outs
    fn = jax.jit(
        shard_map(_body, mesh=mesh, in_specs=in_specs, out_specs=out_specs,
                  check_rep=False),
        keep_unused=True,
    )
    return fn, in_names, out_names, out_avals, zero_outs


def get_runner(reps=1):
    key = ("runner", reps)
    if key not in _CACHE:
        nc = build_nc(8192, reps=reps)
        _CACHE[key] = _make_callable(nc, N_CORES)
    return _CACHE[key]


def run_cores(pred, target, reps=1):
    """pred/target: [8, 8192, 3] f32 -> per-core res arrays [8, 128, 2]."""
    fn, in_names, out_names, out_avals, zero_outs = get_runner(reps)
    ident = np.eye(128, dtype=ml_dtypes.bfloat16)
    per_core = {
        "pred": [np.ascontiguousarray(pred[b]) for b in range(N_CORES)],
        "target": [np.ascontiguousarray(target[b]) for b in range(N_CORES)],
        "ident": [ident] * N_CORES,
    }
    concat_in = [np.concatenate(per_core[name], axis=0) for name in in_names]
    concat_zero = [np.zeros((N_CORES * z.shape[0], *z.shape[1:]), z.dtype)
                   for z in zero_outs]
    outs = fn(*concat_in, *concat_zero)
    res = np.asarray(outs[out_names.index("res")]).reshape(N_CORES, 128, 2)
    return res


def kernel(pred, target):
    pred = np.asarray(pred, dtype=np.float32)
    target = np.asarray(target, dtype=np.float32)
    res = run_cores(pred, target)
    r = res.astype(np.float64)
    n = float(pred.shape[0] * pred.shape[1])
    loss = (r[:, :, 0].sum() + r[:, :, 1].sum()) / n
    return np.float32(loss)
